# revision 11
# baseline (speedup 1.0000x reference)
"""Trainium2 Bass kernel for edge-biased multi-head attention.

Reference computation (B=2, S=384, DM=512, NH=8, DK=64):
    qh/kh/vh = per-head projections of q/k/v
    scores[b,h,i,j] = (qh . kh + qh . EK[b,j,i,(h,:)]) / sqrt(dk)
    scores masked where mask[b,i]==1  (whole query row -> uniform softmax)
    attn = softmax_j(scores)
    out_h[b,h,i,:] = sum_j attn * (vh[b,h,j,:] + EV[b,j,i,(h,:)])
    out = concat(out_h) @ Wo.T + bo

Sharding: 8 cores = 2 batches x 4 query-row blocks of 96. Each core owns
(b, i0:i0+96) end to end (softmax is core-local); host concatenates rows.

Per-core pipeline:
  - pass A: stream EK[b, :, iblk, :] in j-blocks (fp32->bf16 cast DMA),
    DVE multiply against broadcast qh + pairwise-add tree for the per-head
    segment reduction; PE adds the qh.khT base scores.
  - softmax batched over the [96, 8, 384] score tile; mask folded into the
    exp scale (masked rows -> exp(0) -> uniform), 1/sqrt(dk) folded there too.
  - pass B: PE-transpose attn to [j, (i,h)]; vh term = 24 batched matmuls;
    EV term = 3 matmuls per query row with EV[b,:,i,:] streamed in natural
    [j, hd] layout; diagonal blocks extracted with small SBUF->SBUF DMAs.
  - output projection on PE, + bo, DMA out.
"""

import warnings

warnings.filterwarnings("ignore")

from contextlib import ExitStack

import numpy as np

import concourse.bass as bass
import concourse.mybir as mybir
import concourse.tile as tile
from concourse import masks
from concourse.bass_utils import run_bass_kernel_spmd

F32 = mybir.dt.float32
BF16 = mybir.dt.bfloat16
I32 = mybir.dt.int32
ALU = mybir.AluOpType
ACTF = mybir.ActivationFunctionType

B, S, DM, NH, DK = 2, 384, 512, 8, 64
NCORES = 8
IBLK = 96          # query rows per core
JB = 8             # j-block size for the EK stream
NJB = S // JB      # 48
JC = S // 128      # 3 j partition-chunks
KC = DM // 128     # 4 contraction chunks
DIAG_G = 12        # i rows per diagonal-extract group


def _split_multi_waits(nc, max_inline=1):
    """Walrus in this container rejects >1 inline sync-wait per instruction.
    Hoist extras into standalone single-wait EventSemaphore instructions."""
    uid = 0
    for f in nc.m.functions:
        for blk in f.blocks:
            new_insts = []
            for inst in blk.instructions:
                si = inst.sync_info
                waits = list(si.on_wait) if si is not None else []
                if len(waits) > max_inline:
                    keep = waits[-max_inline:]
                    for w in waits[: len(waits) - max_inline]:
                        uid += 1
                        new_insts.append(
                            mybir.InstEventSemaphore(
                                name=f"waitsplit_{uid}_{inst.name}",
                                engine=inst.engine,
                                ins=[],
                                outs=[],
                                sync_info=mybir.SyncInfo(on_wait=[w], on_update=[]),
                            )
                        )
                    inst.sync_info = mybir.SyncInfo(
                        on_wait=keep, on_update=list(si.on_update)
                    )
                new_insts.append(inst)
            blk.instructions = new_insts


def build_program(split_waits=True, stage=99):
    nc = bass.Bass("TRN2", target_bir_lowering=False, debug=False)

    qT = nc.dram_tensor("qT", [DM, IBLK], F32, kind="ExternalInput")
    kT = nc.dram_tensor("kT", [DM, S], F32, kind="ExternalInput")
    vT = nc.dram_tensor("vT", [DM, S], F32, kind="ExternalInput")
    ek = nc.dram_tensor("ek", [S, IBLK, DM], F32, kind="ExternalInput")
    ev = nc.dram_tensor("ev", [S, IBLK, DM], F32, kind="ExternalInput")
    msk = nc.dram_tensor("msk", [IBLK, 1], F32, kind="ExternalInput")
    wqT = nc.dram_tensor("wqT", [DM, DM], F32, kind="ExternalInput")
    wkT = nc.dram_tensor("wkT", [DM, DM], F32, kind="ExternalInput")
    wvT = nc.dram_tensor("wvT", [DM, DM], F32, kind="ExternalInput")
    woT = nc.dram_tensor("woT", [DM, DM], F32, kind="ExternalInput")
    bqv = nc.dram_tensor("bqv", [1, DM], F32, kind="ExternalInput")
    bkv = nc.dram_tensor("bkv", [1, DM], F32, kind="ExternalInput")
    bvv = nc.dram_tensor("bvv", [1, DM], F32, kind="ExternalInput")
    bov = nc.dram_tensor("bov", [1, DM], F32, kind="ExternalInput")
    y = nc.dram_tensor("y", [IBLK, DM], F32, kind="ExternalOutput")

    with tile.TileContext(nc) as tc:
        with ExitStack() as ctx:
            _body(nc, tc, ctx, locals(), stage)
    if split_waits:
        _split_multi_waits(nc)
    return nc


def _body(nc, tc, ctx, t, stage=99):
    qT, kT, vT, ek, ev, msk = t["qT"], t["kT"], t["vT"], t["ek"], t["ev"], t["msk"]
    wqT, wkT, wvT, woT = t["wqT"], t["wkT"], t["wvT"], t["woT"]
    bqv, bkv, bvv, bov = t["bqv"], t["bkv"], t["bvv"], t["bov"]
    y = t["y"]

    const = ctx.enter_context(tc.tile_pool(name="const", bufs=1))
    persist = ctx.enter_context(tc.tile_pool(name="persist", bufs=1))
    ps_acc = ctx.enter_context(tc.tile_pool(name="ps_acc", bufs=1, space="PSUM"))
    ps_work = ctx.enter_context(tc.tile_pool(name="ps_work", bufs=3, space="PSUM"))
    ps_o2 = ctx.enter_context(tc.tile_pool(name="ps_o2", bufs=2, space="PSUM"))
    stream = ctx.enter_context(tc.tile_pool(name="stream", bufs=2))
    work = ctx.enter_context(tc.tile_pool(name="work", bufs=2))

    # ---- constants ----
    id_f32 = const.tile([128, 128], F32)
    masks.make_identity(nc, id_f32[:])
    id_bf16 = const.tile([128, 128], BF16)
    masks.make_identity(nc, id_bf16[:])

    # ---- load weights / inputs (setup) ----
    wq_sb = persist.tile([128, KC, DM], F32, tag="wq")
    wk_sb = persist.tile([128, KC, DM], F32, tag="wk")
    wv_sb = persist.tile([128, KC, DM], F32, tag="wv")
    wo_sb = persist.tile([128, KC, DM], F32, tag="wo")
    for w_sb, w_dr in ((wq_sb, wqT), (wk_sb, wkT), (wv_sb, wvT), (wo_sb, woT)):
        nc.gpsimd.dma_start(
            out=w_sb[:], in_=w_dr.ap().rearrange("(c p) m -> p c m", p=128)
        )
    qT_sb = persist.tile([128, KC, IBLK], F32, tag="qT")
    nc.gpsimd.dma_start(out=qT_sb[:], in_=qT.ap().rearrange("(c p) m -> p c m", p=128))
    kT_sb = persist.tile([128, KC, S], F32, tag="kT")
    nc.gpsimd.dma_start(out=kT_sb[:], in_=kT.ap().rearrange("(c p) m -> p c m", p=128))
    vT_sb = persist.tile([128, KC, S], F32, tag="vT")
    nc.gpsimd.dma_start(out=vT_sb[:], in_=vT.ap().rearrange("(c p) m -> p c m", p=128))

    mask_sb = persist.tile([IBLK, 1], F32, tag="mask")
    nc.gpsimd.dma_start(out=mask_sb[:], in_=msk.ap())

    # biases broadcast across partitions
    bq_b = persist.tile([IBLK, DM], F32, tag="bq")
    bo_b = persist.tile([IBLK, DM], F32, tag="bo")
    bv_b = persist.tile([128, DM], F32, tag="bv")
    bk_c = persist.tile([128, KC], F32, tag="bkc")  # bk[kc*128+p] per column
    for dst, src in ((bq_b, bqv), (bo_b, bov), (bv_b, bvv)):
        nparts = dst.shape[0] if hasattr(dst, "shape") else 128
        nc.gpsimd.dma_start(
            out=dst[:], in_=src.ap().broadcast_to((dst[:].shape[0], DM))
        )
    nc.gpsimd.dma_start(
        out=bk_c[:], in_=bkv.ap().rearrange("o (c p) -> (o p) c", p=128)
    )

    # ---- projections ----
    # qh [96, 512] = q @ Wq.T + bq
    qh_ps = ps_work.tile([IBLK, DM], F32, tag="w")
    for kc in range(KC):
        nc.tensor.matmul(
            qh_ps[:], qT_sb[:, kc, :], wq_sb[:, kc, :],
            start=(kc == 0), stop=(kc == KC - 1),
        )
    qh_f32 = persist.tile([IBLK, DM], F32, tag="qh32")
    nc.vector.tensor_tensor(out=qh_f32[:], in0=qh_ps[:], in1=bq_b[:], op=ALU.add)
    qh_bf = persist.tile([IBLK, DM], BF16, tag="qhbf")
    nc.vector.tensor_copy(qh_bf[:], qh_f32[:])

    # qhT [hd, i] (4 chunks) via PE transpose of qh
    qhT_sb = persist.tile([128, KC, IBLK], F32, tag="qhT")
    for kc in range(KC):
        tp = ps_work.tile([128, IBLK], F32, tag="w")
        nc.tensor.transpose(
            tp[:], qh_f32[:, kc * 128 : (kc + 1) * 128], id_f32[:IBLK, :IBLK]
        )
        nc.vector.tensor_copy(qhT_sb[:, kc, :], tp[:])

    # khT [hd, j] (4 chunks): khT = Wk @ kT (+ bk per-partition)
    khT_sb = persist.tile([128, KC, S], F32, tag="khT")
    for mc in range(KC):
        kh_ps = ps_work.tile([128, S], F32, tag="w")
        for kc in range(KC):
            nc.tensor.matmul(
                kh_ps[:], wk_sb[:, kc, mc * 128 : (mc + 1) * 128], kT_sb[:, kc, :],
                start=(kc == 0), stop=(kc == KC - 1),
            )
        nc.scalar.activation(
            khT_sb[:, mc, :], kh_ps[:], ACTF.Identity, bias=bk_c[:, mc : mc + 1]
        )

    # vh [j, hd] bf16 (3 j-chunks): vh = v @ Wv.T + bv
    vh_sb = persist.tile([128, JC, DM], BF16, tag="vh")
    for jc in range(JC):
        vh_ps = ps_work.tile([128, DM], F32, tag="w")
        for kc in range(KC):
            nc.tensor.matmul(
                vh_ps[:], vT_sb[:, kc, jc * 128 : (jc + 1) * 128], wv_sb[:, kc, :],
                start=(kc == 0), stop=(kc == KC - 1),
            )
        nc.vector.tensor_tensor(out=vh_sb[:, jc, :], in0=vh_ps[:], in1=bv_b[:], op=ALU.add)

    def _dump(ap96x512):
        y_dbg = persist.tile([IBLK, DM], F32, tag="ydbg")
        nc.vector.tensor_copy(y_dbg[:], ap96x512)
        nc.sync.dma_start(out=y.ap(), in_=y_dbg[:])

    if stage <= 1:
        _dump(qh_f32[:])
        return

    # ---- pass A: scores ----
    scores = persist.tile([IBLK, NH, S], F32, tag="scores")
    # base scores qh . khT
    for h in range(NH):
        pl = (h % 2) * 64
        kc = h // 2
        sc_ps = ps_work.tile([IBLK, S], F32, tag="w")
        nc.tensor.matmul(
            sc_ps[:],
            qhT_sb[pl : pl + 64, kc, :],
            khT_sb[pl : pl + 64, kc, :],
            start=True, stop=True,
        )
        nc.scalar.copy(scores[:, h, :], sc_ps[:])

    # EK bias scores
    for blk in range(NJB):
        j0 = blk * JB
        ek_t = stream.tile([IBLK, JB, DM], BF16, tag="ekt")
        nc.gpsimd.dma_start(
            out=ek_t[:],
            in_=ek.ap()[j0 : j0 + JB].rearrange("j i d -> i j d"),
        )
        tt = work.tile([IBLK, JB, NH, DK], BF16, tag="tt")
        nc.vector.tensor_tensor(
            out=tt[:],
            in0=ek_t[:].rearrange("p j (h d) -> p j h d", d=DK),
            in1=qh_bf[:]
            .rearrange("p (h d) -> p h d", d=DK)
            .unsqueeze(1)
            .broadcast_to((IBLK, JB, NH, DK)),
            op=ALU.mult,
        )
        t1 = work.tile([IBLK, JB, NH, 32], BF16, tag="t1")
        nc.vector.tensor_tensor(
            out=t1[:], in0=tt[:, :, :, 0:32], in1=tt[:, :, :, 32:64], op=ALU.add
        )
        t2 = work.tile([IBLK, JB, NH, 16], BF16, tag="t2")
        nc.vector.tensor_tensor(
            out=t2[:], in0=t1[:, :, :, 0:16], in1=t1[:, :, :, 16:32], op=ALU.add
        )
        t3 = work.tile([IBLK, JB, NH, 8], BF16, tag="t3")
        nc.vector.tensor_tensor(
            out=t3[:], in0=t2[:, :, :, 0:8], in1=t2[:, :, :, 8:16], op=ALU.add
        )
        t4 = work.tile([IBLK, JB, NH, 4], F32, tag="t4")
        nc.vector.tensor_tensor(
            out=t4[:], in0=t3[:, :, :, 0:4], in1=t3[:, :, :, 4:8], op=ALU.add
        )
        t5 = work.tile([IBLK, JB, NH, 2], F32, tag="t5")
        nc.vector.tensor_tensor(
            out=t5[:], in0=t4[:, :, :, 0:2], in1=t4[:, :, :, 2:4], op=ALU.add
        )
        t6 = work.tile([IBLK, JB, NH], F32, tag="t6")
        nc.vector.tensor_tensor(
            out=t6[:], in0=t5[:, :, :, 0], in1=t5[:, :, :, 1], op=ALU.add
        )
        sl = scores[:, :, j0 : j0 + JB].rearrange("p h j -> p j h")
        nc.vector.tensor_tensor(out=sl, in0=t6[:], in1=sl, op=ALU.add)

    if stage <= 2:
        _dump(scores[:].rearrange("p h j -> p (h j)")[:, 0:DM])
        return

    # ---- softmax over j (mask + 1/sqrt(dk) folded into exp scale) ----
    maskf8 = persist.tile([IBLK, 1], F32, tag="maskf8")
    nc.vector.tensor_scalar(
        out=maskf8[:], in0=mask_sb[:], scalar1=-0.125, scalar2=0.125,
        op0=ALU.mult, op1=ALU.add,
    )
    rmax = persist.tile([IBLK, NH], F32, tag="rmax")
    nc.vector.tensor_reduce(
        out=rmax[:], in_=scores[:], axis=mybir.AxisListType.X, op=ALU.max
    )
    negmax = persist.tile([IBLK, NH], F32, tag="negmax")
    nc.vector.tensor_scalar(
        out=negmax[:], in0=rmax[:], scalar1=maskf8[:], scalar2=-1.0,
        op0=ALU.mult, op1=ALU.mult,
    )
    attn = persist.tile([IBLK, NH, S], BF16, tag="attn")
    ssum = persist.tile([IBLK, NH], F32, tag="ssum")
    for h in range(NH):
        nc.scalar.activation(
            attn[:, h, :], scores[:, h, :], ACTF.Exp,
            bias=negmax[:, h : h + 1], scale=maskf8[:],
            accum_out=ssum[:, h : h + 1],
        )
    rsum = persist.tile([IBLK, NH], F32, tag="rsum")
    nc.vector.reciprocal(rsum[:], ssum[:])
    for h in range(NH):
        nc.vector.tensor_scalar(
            out=attn[:, h, :], in0=attn[:, h, :], scalar1=rsum[:, h : h + 1],
            scalar2=None, op0=ALU.mult,
        )

    # ---- attn transpose: attnT[j, jc, i, h] ----
    attnT = persist.tile([128, JC, IBLK, NH], BF16, tag="attnT")
    for h in range(NH):
        for jc in range(JC):
            atp = ps_work.tile([128, IBLK], BF16, tag="w")
            nc.tensor.transpose(
                atp[:], attn[:, h, jc * 128 : (jc + 1) * 128], id_bf16[:IBLK, :IBLK]
            )
            nc.vector.tensor_copy(attnT[:, jc, :, h], atp[:])

    if stage <= 3:
        _dump(attnT[:].rearrange("p c i h -> p (c i h)")[:IBLK, 0:DM])
        return

    # ---- pass B ----
    # vh term: out_vh[i, (h,d)] = sum_j attn[j,i,h] * vh[j, (h,d)]
    out_vh = ps_acc.tile([IBLK, DM], F32, tag="outvh")
    for h in range(NH):
        for jc in range(JC):
            nc.tensor.matmul(
                out_vh[:, h * DK : (h + 1) * DK],
                attnT[:, jc, :, h],
                vh_sb[:, jc, h * DK : (h + 1) * DK],
                start=(jc == 0), stop=(jc == JC - 1),
            )

    # EV term, 96 rows; diagonal blocks of [8, 512] per-row results
    ctx_sb = persist.tile([IBLK, DM], F32, tag="ctx")
    o2_sb = work.tile([NH, DIAG_G, DM], F32, tag="o2sb")
    for i in range(IBLK):
        ev_t = stream.tile([128, JC, DM], BF16, tag="evt")
        nc.gpsimd.dma_start(
            out=ev_t[:],
            in_=ev.ap()[:, i, :].rearrange("(c p) d -> p c d", p=128),
        )
        o2 = ps_o2.tile([NH, DM], F32, tag="o2")
        for jc in range(JC):
            nc.tensor.matmul(
                o2[:], attnT[:, jc, i, :], ev_t[:, jc, :],
                start=(jc == 0), stop=(jc == JC - 1),
            )
        g = i % DIAG_G
        nc.scalar.copy(o2_sb[:, g, :], o2[:])
        if g == DIAG_G - 1:
            g0 = i - DIAG_G + 1
            for h in range(NH):
                nc.sync.dma_start(
                    out=ctx_sb[g0 : g0 + DIAG_G, h * DK : (h + 1) * DK],
                    in_=o2_sb[h : h + 1, :, h * DK : (h + 1) * DK],
                )
            if i != IBLK - 1:
                o2_sb = work.tile([NH, DIAG_G, DM], F32, tag="o2sb")

    if stage <= 4:
        _dump(ctx_sb[:])
        return

    nc.vector.tensor_tensor(out=ctx_sb[:], in0=ctx_sb[:], in1=out_vh[:], op=ALU.add)

    if stage <= 5:
        _dump(ctx_sb[:])
        return

    # ---- output projection ----
    ctxT = persist.tile([128, KC, IBLK], F32, tag="ctxT")
    for kc in range(KC):
        cxp = ps_work.tile([128, IBLK], F32, tag="w")
        nc.tensor.transpose(
            cxp[:], ctx_sb[:, kc * 128 : (kc + 1) * 128], id_f32[:IBLK, :IBLK]
        )
        nc.vector.tensor_copy(ctxT[:, kc, :], cxp[:])

    if stage <= 6:
        _dump(ctxT[:].rearrange("p c i -> p (c i)")[:IBLK, 0:DM])
        return

    y_ps = ps_work.tile([IBLK, DM], F32, tag="w")
    for kc in range(KC):
        nc.tensor.matmul(
            y_ps[:], ctxT[:, kc, :], wo_sb[:, kc, :],
            start=(kc == 0), stop=(kc == KC - 1),
        )
    y_sb = persist.tile([IBLK, DM], F32, tag="ysb")
    nc.vector.tensor_tensor(out=y_sb[:], in0=y_ps[:], in1=bo_b[:], op=ALU.add)
    nc.sync.dma_start(out=y.ap(), in_=y_sb[:])


_NC_CACHE = {}


def _get_nc():
    if "nc" not in _NC_CACHE:
        _NC_CACHE["nc"] = build_program()
    return _NC_CACHE["nc"]


def make_in_maps(q, k, v, edge_bias_k, edge_bias_v, mask, Wq, bq, Wk, bk, Wv, bv, Wo, bo):
    f = np.float32
    c = np.ascontiguousarray
    wqT, wkT, wvT, woT = (c(W.T.astype(f)) for W in (Wq, Wk, Wv, Wo))
    in_maps = []
    for core in range(NCORES):
        b = core // (NCORES // B)
        i0 = (core % (NCORES // B)) * IBLK
        in_maps.append(
            {
                "qT": c(q[b, i0 : i0 + IBLK, :].T.astype(f)),
                "kT": c(k[b].T.astype(f)),
                "vT": c(v[b].T.astype(f)),
                "ek": c(edge_bias_k[b, :, i0 : i0 + IBLK, :].astype(f)),
                "ev": c(edge_bias_v[b, :, i0 : i0 + IBLK, :].astype(f)),
                "msk": c(mask[b, i0 : i0 + IBLK].astype(f)[:, None]),
                "wqT": wqT, "wkT": wkT, "wvT": wvT, "woT": woT,
                "bqv": c(bq.astype(f)[None, :]),
                "bkv": c(bk.astype(f)[None, :]),
                "bvv": c(bv.astype(f)[None, :]),
                "bov": c(bo.astype(f)[None, :]),
            }
        )
    return in_maps


def kernel(q, k, v, use_qb, edge_bias_k, edge_bias_v, mask, Wq, bq, Wk, bk, Wv, bv, Wo, bo,
           _trace=False):
    q, k, v = np.asarray(q), np.asarray(k), np.asarray(v)
    edge_bias_k, edge_bias_v = np.asarray(edge_bias_k), np.asarray(edge_bias_v)
    mask = np.asarray(mask)
    in_maps = make_in_maps(
        q, k, v, edge_bias_k, edge_bias_v, mask,
        np.asarray(Wq), np.asarray(bq), np.asarray(Wk), np.asarray(bk),
        np.asarray(Wv), np.asarray(bv), np.asarray(Wo), np.asarray(bo),
    )
    nc = _get_nc()
    res = run_bass_kernel_spmd(nc, in_maps, core_ids=list(range(NCORES)), trace=_trace)
    out = np.zeros((B, S, DM), np.float32)
    for core in range(NCORES):
        b = core // (NCORES // B)
        i0 = (core % (NCORES // B)) * IBLK
        out[b, i0 : i0 + IBLK, :] = res.results[core]["y"]
    kernel.last_results = res
    return out
